# revision 71
# baseline (speedup 1.0000x reference)
"""GQA attention prefill (Qwen3-style) on 8 TRN2 NeuronCores.

Sharding: core c -> batch b = c // 4, kv-head pair j = c % 4
  (kv heads {2j, 2j+1}, q heads {4j..4j+3}).
Per core: fused QKV projection (fp16 matmuls, fp32 PSUM; the first two
row tiles interleave k-major with the input DMA stream, preceded by
dummy warm-up matmuls that ramp the PE p-state), per-head RMSNorm
(one ACT Square + one DVE 3-D reduce per tile, rstd applied as a
single stride-0-broadcast multiply) + RoPE (norm weights and the
1/sqrt(HD) score scale are folded into host-side cos/sin tables),
PE transposes into K^T/Q^T layouts, attention with transposed scores
(scoresT[t, s], triple-buffered in PSUM). The causal mask is additive
(-1e5 on the diagonal score block, before exp). exp runs on the scalar
engine into fp16 P tiles, except ~5 cached tiles per head offloaded to
the vector engine via a Schraudolph bit-trick: P = bitcast_f16(
int16(score * 1024*log2(e) + 15315.5)), ~3% per-element error, well
inside the softmax averaging budget. PV accumulates in PSUM trailing
~10 tiles behind QK; the softmax denominator is an online fp16
pairwise tree (DVE, range-trimmed with valid-from tracking; some
level-1 adds on gpsimd), reduced/broadcast per head by gpsimd
partition_all_reduce + a DVE reciprocal, with the per-head normalize
deferred into the next head's tile loop. fp16 output projection and
fp16 output DMA. Host sums the 4 partial outputs per batch (the
o-projection all-reduce) in fp32.
"""

import numpy as np

B, S, HID = 2, 1024, 1024
NH, NKV, HD = 16, 8, 128
G = NH // NKV
CACHE_LEN, MAX_CACHE = 3072, 4096
T = CACHE_LEN + S                  # 4096
N_TT = T // 128                    # 32 t-tiles
N_CT = CACHE_LEN // 128            # 24 cached t-tiles
THETA = 1000000.0
EPS = 1e-6

_STATE = {}


def _build():
    import concourse.tile as tile
    from concourse import bacc, mybir

    f32 = mybir.dt.float32
    f16 = mybir.dt.float16
    i16 = mybir.dt.int16
    AF = mybir.ActivationFunctionType
    OP = mybir.AluOpType

    nc = bacc.Bacc("TRN2", target_bir_lowering=False, debug=False, num_devices=8)

    xt_d = nc.dram_tensor("xt", [128, 8, 1024], f16, kind="ExternalInput").ap()
    wq_d = nc.dram_tensor("wq", [128, 8, 1024], f16, kind="ExternalInput").ap()
    kc_d = nc.dram_tensor("kc", [128, 2, CACHE_LEN], f16, kind="ExternalInput").ap()
    vc_d = nc.dram_tensor("vc", [128, N_CT, 2, 128], f16, kind="ExternalInput").ap()
    cq_d = nc.dram_tensor("cq", [128, 8, 128], f16, kind="ExternalInput").ap()
    sq_d = nc.dram_tensor("sq", [128, 8, 128], f16, kind="ExternalInput").ap()
    ck_d = nc.dram_tensor("ck", [128, 8, 128], f16, kind="ExternalInput").ap()
    sk_d = nc.dram_tensor("sk", [128, 8, 128], f16, kind="ExternalInput").ap()
    wo_d = nc.dram_tensor("wo", [128, 4, 1024], f16, kind="ExternalInput").ap()
    tri_d = nc.dram_tensor("tri", [128, 128], f32, kind="ExternalInput").ap()
    one_d = nc.dram_tensor("one", [128, 1], f16, kind="ExternalInput").ap()
    idn_d = nc.dram_tensor("idn", [128, 128], f16, kind="ExternalInput").ap()
    out_d = nc.dram_tensor("out", [S, HID], f16, kind="ExternalOutput").ap()

    with tile.TileContext(nc) as tc:
        with tc.tile_pool(name="persist", bufs=1) as persist:
            kT = persist.tile([128, 2, T], f16, tag="kT")        # [d, kv, t]
            vT = persist.tile([128, N_TT, 2, 128], f16, tag="vT")  # [tp, ti, kv, d]
            qT = persist.tile([128, 4, S], f16, tag="qT")        # [d, h, s]
            ctx = persist.tile([128, 4, S], f16, tag="ctx")      # [d, h, s]
            wo_sb = persist.tile([128, 4, 1024], f16, tag="wo")
            tri_sb = persist.tile([128, 128], f32, tag="tri")
            one_sb = persist.tile([128, 1], f16, tag="one")
            idn_sb = persist.tile([128, 128], f16, tag="idn")

            # ---------------- Phase 1: QKV projection + norm + rope ----------
            with tc.tile_pool(name="ph1", bufs=1) as ph1, \
                 tc.tile_pool(name="qkp", bufs=3) as qkp, \
                 tc.tile_pool(name="tmp", bufs=4) as tmp, \
                 tc.tile_pool(name="stat", bufs=8) as statp, \
                 tc.tile_pool(name="ps1", bufs=3, space="PSUM") as ps1, \
                 tc.tile_pool(name="pstp", bufs=2, space="PSUM") as pstp:
                xt_sb = ph1.tile([128, 8, 1024], f16, tag="xt")
                wq_sb = ph1.tile([128, 8, 1024], f16, tag="wqkv")
                cq_sb = ph1.tile([128, 8, 128], f16, tag="cq")
                sq_sb = ph1.tile([128, 8, 128], f16, tag="sq")
                ck_sb = ph1.tile([128, 8, 128], f16, tag="ck")
                sk_sb = ph1.tile([128, 8, 128], f16, tag="sk")
                # One explicit ACT table load (set 6 = natural_log_exp_and_
                # others, covers Copy/Ln/Exp/Square) at t=0, overlapped with
                # the input DMAs; bacc's fixpoint pass then sees every
                # activation's table already loaded on all paths.
                eps_t = ph1.tile([128, 1], f32, tag="eps")
                zero_t = ph1.tile([128, 1], f32, tag="zero")
                nc.vector.memset(eps_t[:], EPS)
                nc.vector.memset(zero_t[:], 0.0)
                nc.scalar.add_instruction(mybir.InstLoadActFuncSet(
                    name=nc.get_next_instruction_name(), ins=[], outs=[],
                    act_func_set_id=6))
                # PE warm-up: dummy matmuls on a memset tile ramp the
                # tensor engine to full clock before the first real matmul
                wrm = ph1.tile([128, 512], f16, tag="wrm")
                nc.vector.memset(wrm[:], 0.0)
                # interleave xt/wq per k-tile so the first matmuls start early
                for kt in range(8):
                    nc.sync.dma_start(out=xt_sb[:, kt, :], in_=xt_d[:, kt, :])
                    nc.sync.dma_start(out=wq_sb[:, kt, :], in_=wq_d[:, kt, :])
                nc.sync.dma_start(out=cq_sb[:], in_=cq_d[:])
                nc.sync.dma_start(out=sq_sb[:], in_=sq_d[:])
                nc.sync.dma_start(out=ck_sb[:], in_=ck_d[:])
                nc.sync.dma_start(out=sk_sb[:], in_=sk_d[:])
                nc.sync.dma_start(out=idn_sb[:], in_=idn_d[:])
                for tch in range(4):
                    nc.sync.dma_start(
                        out=kT[:, :, 768 * tch:768 * (tch + 1)],
                        in_=kc_d[:, :, 768 * tch:768 * (tch + 1)])
                nc.sync.dma_start(out=tri_sb[:], in_=tri_d[:])
                nc.sync.dma_start(out=one_sb[:], in_=one_d[:])
                nc.sync.dma_start(out=vT[:, 0:N_CT, :, :], in_=vc_d[:])
                nc.sync.dma_start(out=wo_sb[:], in_=wo_d[:])

                # 4-stage software pipeline over m: each engine's stream
                # always has ready work (strict per-engine program order).
                stA = {}

                def stage_a_mm(m, ps):
                    for c in range(2):
                        for kt in range(8):
                            nc.tensor.matmul(
                                ps[:, 512 * c:512 * c + 512],
                                lhsT=xt_sb[:, kt, 128 * m:128 * m + 128],
                                rhs=wq_sb[:, kt, 512 * c:512 * c + 512],
                                start=(kt == 0),
                                stop=(kt == 7),
                            )

                def stage_a_mm01():
                    # first two m-tiles interleaved kt-major: each arriving
                    # xt/wq k-tile DMA immediately feeds both PSUM groups, so
                    # the DMA lead-in isn't serialized behind m=0 alone
                    ps0 = ps1.tile([128, 1024], f32, tag="qkvps", name="qkvps0")
                    ps1_ = ps1.tile([128, 1024], f32, tag="qkvps", name="qkvps1")
                    for _ in range(10):
                        nc.tensor.matmul(ps0[:, 0:512], lhsT=wrm[:, 0:128],
                                         rhs=wrm[:], start=True, stop=True,
                                         skip_group_check=True)
                    for kt in range(8):
                        for ps in (ps0, ps1_):
                            for c in range(2):
                                nc.tensor.matmul(
                                    ps[:, 512 * c:512 * c + 512],
                                    lhsT=xt_sb[:, kt, (0 if ps is ps0 else 128):
                                               (128 if ps is ps0 else 256)],
                                    rhs=wq_sb[:, kt, 512 * c:512 * c + 512],
                                    start=(kt == 0),
                                    stop=(kt == 7),
                                )
                    return ps0, ps1_

                def stage_a(m, ps=None):  # stats + v copy (ACT/DVE split)
                    if ps is None:
                        ps = ps1.tile([128, 1024], f32, tag="qkvps",
                                      name=f"qkvps{m}")
                        stage_a_mm(m, ps)
                    nc.scalar.copy(
                        out=vT[:, N_CT + m, :, :],
                        in_=ps[:, 768:1024].rearrange("p (a b) -> p a b", a=2),
                    )
                    sq6 = tmp.tile([128, 768], f32, tag="sq6",
                                   name=f"sq6{m}")
                    rstd = statp.tile([128, 6], f32, tag="rstd",
                                      name=f"rstd{m}")
                    nc.scalar.activation(
                        out=sq6[:], in_=ps[:, 0:768], func=AF.Square)
                    nc.vector.tensor_reduce(
                        out=rstd[:], in_=sq6[:].rearrange(
                            "p (h d) -> p h d", h=6),
                        axis=mybir.AxisListType.X, op=OP.add)
                    # rstd = (ms + eps)^-0.5 = exp(-0.5 * ln(ms + eps))
                    nc.scalar.activation(
                        out=rstd[:], in_=rstd[:], func=AF.Ln,
                        bias=eps_t[:], scale=1.0 / HD,
                    )
                    nc.scalar.activation(
                        out=rstd[:], in_=rstd[:], func=AF.Exp,
                        bias=zero_t[:], scale=-0.5,
                    )
                    stA[m] = (ps, rstd)

                def stage_b(m):  # DVE: normalized q/k copies out of PSUM
                    ps, rstd = stA[m]
                    qn = qkp.tile([128, 768], f16, tag="qk", name=f"qn{m}")
                    # one multiply: rstd broadcast along d via stride-0 AP
                    nc.vector.tensor_mul(
                        qn[:].rearrange("p (h d) -> p h d", h=6),
                        ps[:, 0:768].rearrange("p (h d) -> p h d", h=6),
                        rstd[:].unsqueeze(2).broadcast_to((128, 6, 128)),
                    )
                    stA[m] = qn

                def stage_c(m):  # DVE/Pool: rope
                    qn = stA[m]
                    qn4 = qn[:, 0:512].rearrange("p (h d) -> p h d", h=4)
                    qn2 = qn[:, 512:768].rearrange("p (h d) -> p h d", h=2)
                    t1 = tmp.tile([128, 768], f16, tag="t1", name=f"t1_{m}")
                    t2 = tmp.tile([128, 768], f16, tag="t2", name=f"t2_{m}")
                    t1q = t1[:, 0:512].rearrange("p (h d) -> p h d", h=4)
                    t1k = t1[:, 512:768].rearrange("p (h d) -> p h d", h=2)
                    t2q = t2[:, 0:512].rearrange("p (h d) -> p h d", h=4)
                    t2k = t2[:, 512:768].rearrange("p (h d) -> p h d", h=2)
                    cqb = cq_sb[:, m, :].unsqueeze(1).broadcast_to((128, 4, 128))
                    ckb = ck_sb[:, m, :].unsqueeze(1).broadcast_to((128, 2, 128))
                    sqb = sq_sb[:, m, :].unsqueeze(1).broadcast_to((128, 4, 128))
                    skb = sk_sb[:, m, :].unsqueeze(1).broadcast_to((128, 2, 128))
                    nc.vector.tensor_mul(t1q, qn4, cqb)
                    nc.vector.tensor_mul(t1k, qn2, ckb)
                    nc.vector.tensor_mul(
                        t2q[:, :, 0:64], qn4[:, :, 64:128], sqb[:, :, 0:64])
                    nc.vector.tensor_mul(
                        t2q[:, :, 64:128], qn4[:, :, 0:64], sqb[:, :, 64:128])
                    nc.vector.tensor_mul(
                        t2k[:, :, 0:64], qn2[:, :, 64:128], skb[:, :, 0:64])
                    nc.vector.tensor_mul(
                        t2k[:, :, 64:128], qn2[:, :, 0:64], skb[:, :, 64:128])
                    nc.vector.tensor_add(t1[:], t1[:], t2[:])
                    stA[m] = t1

                def stage_d(m):  # PE transposes + copies into qT/kT
                    t1 = stA.pop(m)
                    tp = pstp.tile([128, 768], f16, tag="tp", name=f"tp{m}")
                    for hi in range(6):
                        nc.tensor.transpose(
                            tp[:, 128 * hi:128 * hi + 128],
                            t1[:, 128 * hi:128 * hi + 128], idn_sb[:])
                    nc.scalar.copy(
                        out=qT[:, :, 128 * m:128 * m + 128],
                        in_=tp[:, 0:512].rearrange("p (h d) -> p h d", h=4))
                    nc.scalar.copy(
                        out=kT[:, :, CACHE_LEN + 128 * m:
                               CACHE_LEN + 128 * m + 128],
                        in_=tp[:, 512:768].rearrange("p (h d) -> p h d", h=2))

                # per-step emission order: ready work (b/c of m-1, d of m-2)
                # goes first in each engine's in-order stream; stage_a's
                # stats (which wait on this step's matmuls) go last
                ps01 = stage_a_mm01()
                for step in range(10):
                    if 1 <= step <= 8:
                        stage_b(step - 1)
                        stage_c(step - 1)
                    if step >= 2:
                        stage_d(step - 2)
                    if step < 2:
                        stage_a(step, ps=ps01[step])
                    elif step < 8:
                        stage_a(step)

            # ---------------- Phase 2: attention ----------------------------
            from concourse import bass_isa
            with tc.tile_pool(name="pp", bufs=16) as ppool, \
                 tc.tile_pool(name="bcp", bufs=2) as bcp, \
                 tc.tile_pool(name="ltree", bufs=4) as ltree, \
                 tc.tile_pool(name="sps", bufs=3, space="PSUM") as sps, \
                 tc.tile_pool(name="cps", bufs=1, space="PSUM") as cps:
                # cached-tile exp offload: DVE/Pool compute P via the fp16
                # Schraudolph bit-trick round(s*1024*log2(e) + B16) written
                # as int16 and bitcast to fp16 (max rel err 3.0%; end-to-end
                # ~0.9e-2 vs the 2e-2 budget). Only cached tiles qualify:
                # unmasked, and |score| <= ~6.5 keeps the bits positive.
                A16 = 1477.3197  # 1024 / ln 2
                B16 = 15315.5
                tail_jobs = []
                for h in range(4):
                    # head 3 leans on ACT: its exp tail gates phase 3.
                    # head 0's offload tiles sit late so its early tiles
                    # stream through ACT while phase 1 finishes.
                    # (gpsimd cannot read PSUM, so only DVE offloads exp)
                    SCH_DVE = ({3, 9} if h == 3 else
                               {3, 7, 11, 15, 19} if h == 0 else
                               {3, 7, 11, 15, 19, 23})
                    SCH_POOL = set()
                    kv = h // 2
                    ctx_ps = cps.tile([128, S], f32, tag="ctxps",
                                      name=f"ctxps{h}")

                    def pv(i, s_lo, psl):
                        for c in range(2):
                            c_lo, c_hi = max(s_lo, 512 * c), 512 * (c + 1)
                            if c_lo >= c_hi:
                                continue
                            last_i = N_CT + 4 * (c + 1) - 1
                            nc.tensor.matmul(
                                ctx_ps[:, c_lo:c_hi],
                                lhsT=vT[:, i, kv, :],
                                rhs=psl(slice(c_lo, c_hi)),
                                start=(i == 0), stop=(i == last_i),
                            )

                    # online pairwise tree for l[s] = sum_t P[t, s]: fp16
                    # adds on DVE (2x mode) replace a PE ones-matmul stream
                    levels = [None] * 6
                    pend = []
                    for i in range(N_TT):
                        s_lo = max(0, 128 * (i - N_CT))
                        # PV trails so PE fills the exp latency with the
                        # next tiles' QK matmuls.
                        if len(pend) > (10 if i < 14 else 6):
                            pv(*pend.pop(0))
                        sc = sps.tile([128, S], f32, tag="sc", name=f"sc{h}_{i}")
                        for c in range(2):
                            c_lo, c_hi = max(s_lo, 512 * c), 512 * (c + 1)
                            if c_lo >= c_hi:
                                continue
                            nc.tensor.matmul(
                                sc[:, c_lo:c_hi],
                                lhsT=kT[:, kv, 128 * i:128 * i + 128],
                                rhs=qT[:, h, c_lo:c_hi],
                                start=True, stop=True,
                            )
                        if i in SCH_DVE or i in SCH_POOL:
                            Pi = ppool.tile([128, S], i16, tag="Pi")
                            eng = nc.vector if i in SCH_DVE else nc.gpsimd
                            eng.tensor_scalar(
                                out=Pi[:], in0=sc[:], scalar1=A16,
                                scalar2=B16, op0=OP.mult, op1=OP.add)
                            psl = (lambda Pi: lambda sl: Pi[:, sl].bitcast(f16))(Pi)
                        else:
                            P_t = ppool.tile([128, S], f16, tag="P")
                            if i >= N_CT:
                                # additive causal mask on the diagonal score
                                # block (-1e5 strictly below the diagonal):
                                # exp then produces exact zeros, so no
                                # separate P masking pass is needed
                                nc.vector.tensor_add(
                                    sc[:, s_lo:s_lo + 128],
                                    sc[:, s_lo:s_lo + 128],
                                    tri_sb[:],
                                )
                            nc.scalar.activation(
                                out=P_t[:, s_lo:S], in_=sc[:, s_lo:S],
                                func=AF.Exp,
                            )
                            psl = (lambda P_t: lambda sl: P_t[:, sl])(P_t)
                        pend.append((i, s_lo, psl))
                        # previous head's l/normalize tail drains here (after
                        # this tile's QK+exp) so the boundary never serializes
                        if tail_jobs:
                            tail_jobs.pop(0)()
                        # leaves are valid from s_lo; internal nodes from
                        # their own written range. The parent copies the left
                        # child across the gap.
                        cur, lo, vf, k = psl, s_lo, s_lo, 0
                        while levels[k] is not None:
                            nxt = ltree.tile([128, S], f16, tag=f"lv{k + 1}",
                                             name=f"lv{k + 1}_{h}_{i}")
                            left, plo = levels[k]
                            if vf > plo:
                                nc.vector.tensor_copy(
                                    out=nxt[:, plo:vf],
                                    in_=left(slice(plo, vf)))
                            # gpsimd takes alternate level-1 adds of cached
                            # pairs; everything else stays on the 2x DVE
                            eng = (nc.gpsimd if k == 0 and i < N_CT
                                   and (i % 4 == 1) else nc.vector)
                            eng.tensor_add(
                                nxt[:, vf:S], left(slice(vf, S)),
                                cur(slice(vf, S)))
                            levels[k] = None
                            nxt_sl = (lambda t: lambda sl: t[:, sl])(nxt)
                            cur, lo, vf, k = nxt_sl, plo, plo, k + 1
                        levels[k] = (cur, lo)

                    # softmax denominator: gpsimd partition_all_reduce of the
                    # tree root directly yields the broadcast l, then a DVE
                    # reciprocal + per-chunk normalization. Deferred into the
                    # next head's loop (except head 3, which phase 3 needs).
                    def make_tail(h, kv, ctx_ps, root, pend, pv=pv):
                        for j in pend:
                            pv(*j)
                        jobs = []
                        nch = 2 if h == 3 else 1
                        csz = S // nch
                        def seg(cc):
                            def run():
                                sl = slice(csz * cc, csz * cc + csz)
                                bc = bcp.tile([128, S], f32, tag="bc",
                                              name=f"bc{h}_{cc}")
                                nc.gpsimd.partition_all_reduce(
                                    out_ap=bc[:, sl], in_ap=root(sl),
                                    channels=128,
                                    reduce_op=bass_isa.ReduceOp.add)
                                rb = bcp.tile([128, S], f32, tag="rb",
                                              name=f"rb{h}_{cc}")
                                nc.vector.reciprocal_approx_fast(
                                    out=rb[:, sl], in_=bc[:, sl])
                                for c4 in range(csz // 256):
                                    s2 = slice(csz * cc + 256 * c4,
                                               csz * cc + 256 * c4 + 256)
                                    nc.vector.tensor_mul(
                                        ctx[:, h, s2], ctx_ps[:, s2],
                                        rb[:, s2])
                            return run
                        return jobs + [seg(cc) for cc in range(nch)]

                    tail_jobs.extend(
                        make_tail(h, kv, ctx_ps, levels[5][0], pend))
                    if h == 3:
                        while tail_jobs:
                            tail_jobs.pop(0)()

            # ---------------- Phase 3: output projection --------------------
            with tc.tile_pool(name="osb", bufs=3) as osb, \
                 tc.tile_pool(name="ops", bufs=3, space="PSUM") as ops:
                for m in range(8):
                    op = ops.tile([128, 1024], f32, tag="ops", name=f"op{m}")
                    for c2 in range(2):
                        for h2 in range(4):
                            nc.tensor.matmul(
                                op[:, 512 * c2:512 * c2 + 512],
                                lhsT=ctx[:, h2, 128 * m:128 * m + 128],
                                rhs=wo_sb[:, h2, 512 * c2:512 * c2 + 512],
                                start=(h2 == 0), stop=(h2 == 3),
                            )
                    ot = osb.tile([128, 1024], f16, tag="ot", name=f"ot{m}")
                    cp = nc.vector.tensor_copy if m % 2 else nc.scalar.copy
                    cp(out=ot[:], in_=op[:])
                    nc.sync.dma_start(
                        out=out_d[128 * m:128 * m + 128, :], in_=ot[:])

    nc.compile()
    return nc


def _get_nc():
    if "nc" not in _STATE:
        _STATE["nc"] = _build()
    return _STATE["nc"]


def _host_tables(q_norm_w, k_norm_w, cache_len):
    pos = np.arange(cache_len, cache_len + S, dtype=np.float32)
    inv_freq = (1.0 / (THETA ** (np.arange(0, HD, 2, dtype=np.float32) / HD))) \
        .astype(np.float32)
    freqs = pos[:, None] * inv_freq[None, :]          # [S, 64]
    emb = np.concatenate([freqs, freqs], axis=-1)     # [S, HD]
    cos = np.cos(emb).astype(np.float32)
    sin = np.sin(emb).astype(np.float32)

    qs = np.float32(HD ** -0.5)
    cq = cos * q_norm_w[None, :] * qs
    ck = cos * k_norm_w[None, :]
    # rotate_half coefficient tables: out[d<64] += x[d+64] * (-sin[d] * w[d+64])
    #                                 out[d>=64] += x[d-64] * (sin[d] * w[d-64])
    sq = np.empty_like(sin)
    sq[:, :64] = -sin[:, :64] * q_norm_w[None, 64:]
    sq[:, 64:] = sin[:, 64:] * q_norm_w[None, :64]
    sq = sq * qs
    sk = np.empty_like(sin)
    sk[:, :64] = -sin[:, :64] * k_norm_w[None, 64:]
    sk[:, 64:] = sin[:, 64:] * k_norm_w[None, :64]

    def tile8(a):  # [S, 128] -> [128, 8, 128]
        return np.ascontiguousarray(
            a.reshape(8, 128, 128).transpose(1, 0, 2)
        ).astype(np.float16)

    return tile8(cq), tile8(sq), tile8(ck), tile8(sk)


def kernel(hidden_states, qkv_weight, q_norm_w, k_norm_w, o_weight,
           k_cache, v_cache, cache_len):
    from concourse.bass_utils import run_bass_kernel_spmd

    assert int(cache_len) == CACHE_LEN, "kernel compiled for cache_len=3072"
    hs = np.asarray(hidden_states, dtype=np.float32)
    wqkv = np.asarray(qkv_weight, dtype=np.float32)
    qnw = np.asarray(q_norm_w, dtype=np.float32)
    knw = np.asarray(k_norm_w, dtype=np.float32)
    wo = np.asarray(o_weight, dtype=np.float32)
    kc = np.asarray(k_cache, dtype=np.float32)
    vc = np.asarray(v_cache, dtype=np.float32)

    cq, sq, ck, sk = _host_tables(qnw, knw, int(cache_len))
    tri = ((1.0 - np.triu(np.ones((128, 128), np.float32))) * -1e5).astype(np.float32)
    one = np.ones((128, 1), np.float16)
    idn = np.eye(128, dtype=np.float16)

    in_maps = []
    for c in range(8):
        b, j = c // 4, c % 4
        xt = np.ascontiguousarray(
            hs[b].T.reshape(8, 128, S).transpose(1, 0, 2)).astype(np.float16)
        wrows = np.concatenate([
            wqkv[512 * j:512 * j + 512],
            wqkv[2048 + 256 * j:2048 + 256 * j + 256],
            wqkv[3072 + 256 * j:3072 + 256 * j + 256],
        ], axis=0)                                     # [1024, HID]
        wq = np.ascontiguousarray(
            wrows.T.reshape(8, 128, 1024).transpose(1, 0, 2)).astype(np.float16)
        kcc = np.ascontiguousarray(
            kc[b, :CACHE_LEN, 2 * j:2 * j + 2, :].transpose(2, 1, 0)
        ).astype(np.float16)
        vcc = np.ascontiguousarray(
            vc[b, :CACHE_LEN, 2 * j:2 * j + 2, :]
            .reshape(N_CT, 128, 2, 128).transpose(1, 0, 2, 3)
        ).astype(np.float16)
        wot = np.ascontiguousarray(
            wo[:, 512 * j:512 * j + 512].T.reshape(4, 128, 1024)
            .transpose(1, 0, 2)).astype(np.float16)
        in_maps.append({
            "xt": xt, "wq": wq, "kc": kcc, "vc": vcc,
            "cq": cq, "sq": sq, "ck": ck, "sk": sk,
            "wo": wot, "tri": tri, "one": one, "idn": idn,
        })

    nc = _get_nc()
    _STATE["last_in_maps"] = in_maps
    res = run_bass_kernel_spmd(nc, in_maps, core_ids=list(range(8)))
    outs = [np.asarray(res.results[i]["out"], dtype=np.float32)
            for i in range(8)]
    full = np.empty((B, S, HID), np.float32)
    for b in range(B):
        full[b] = outs[4 * b] + outs[4 * b + 1] + outs[4 * b + 2] + outs[4 * b + 3]
    return full

